# revision 9
# baseline (speedup 1.0000x reference)
"""Trainium2 Bass kernel for nn_Anchor3 (segment_reduce): 8-core SPMD, v3.

Per core (125k nodes/branch, bf16 data path):
  - segment-sum on the TensorEngine: host sorts each shard by class and
    deals rows into 32 windows of 128 classes (33 chunks of 128 rows per
    window, zero-padded); per chunk the DVE builds a one-hot [row, cls]
    via is_equal(iota, cls_rel) and the PE contracts rows:
    psumT[64, 128] += sdat_chunk[128, 64].T @ onehot[128, 128]
  - per-branch AllReduce of [64, 4096] f32 partial sums (v's collective and
    attention overlap c's phase A); multiply by host 1/(cnt+eps) -> feaT bf16
  - class-level cross-attention in bf16 (queries sharded 512/core): per-head
    padded weights, 4 heads as row-group matmuls for scores and col-group
    matmuls for attn@V (all concurrent via tile_position), single Exp per key
    chunk (f32 in PSUM -> bf16 out), fused sum-of-exp column, division
    deferred past the V-contraction
  - attention output rows are cast to bf16 and quad-replicated into a
    [512, 256] table slice; per-branch AllGather -> [4096, 256] table
  - output gather: host groups same-class rows into quads; SWDGE dma_gather
    (elem=512B, 1024 idx/call, 4 queues) pulls one 4-row quad per index;
    bulk stores write [128, 32, 256] bf16 pieces; host un-sorts.
Host does index-metadata prep only (sort, counts, quad layout) plus the
bf16 casts that define the kernel's working precision.
"""
import functools
import os

import numpy as np
import ml_dtypes

import concourse.bass as bass
import concourse.bacc as bacc
import concourse.mybir as mybir
import concourse.tile as tile
from concourse import library_config
from concourse.bass_utils import run_bass_kernel_spmd

N_CORES = 8
NV = 1_000_000
VN = 4096          # classes per branch
E = 64
H = 4
HD = 16
SHARD = NV // N_CORES            # 125000

WIN = 32                         # class windows per branch (128 classes each)
SLOT = 4                         # rows per slot (class-padded)
GRP = 128                        # slots per stage-1 group (= 512 rows)
SLOT_CH = GRP * SLOT // 128      # stage-1 chunks per group = 4

GC = 1024                        # gather idx per call (SWDGE ring-safe)
QELEM = 256                      # bf16 elems per gathered quad (512B)
CALLS_PER_PIECE = 4              # 4 gather calls -> one 2MB store
QCH = VN // N_CORES              # 512 query rows per core
TC = VN // 128                   # 32 key chunks in attention

DT = mybir.dt.float32
BF = mybir.dt.bfloat16
I16 = mybir.dt.int16
BF_NP = ml_dtypes.bfloat16


def _pairs_from_caps(caps):
    """Static stage-2 pair list from per-window slot capacities.
    Returns (ns, [(g, w, first, last), ...]) ordered by g then w."""
    off = np.r_[0, np.cumsum(caps)]
    ns_used = int(off[-1])
    ns = ((ns_used + GRP - 1) // GRP) * GRP
    pairs = []
    for g in range(ns // GRP):
        lo, hi = g * GRP, (g + 1) * GRP
        for w in range(len(caps)):
            if off[w] < hi and off[w + 1] > lo:
                pairs.append([g, w])
    firsts, lasts = {}, {}
    for i, (g, w) in enumerate(pairs):
        firsts.setdefault(w, i)
        lasts[w] = i
    return ns, [(g, w, firsts[w] == i, lasts[w] == i)
                for i, (g, w) in enumerate(pairs)]


def _build(nq_slots: int, caps_v: tuple, caps_c: tuple):
    """nq_slots: gather quads per branch (multiple of CALLS_PER_PIECE*GC)."""
    skip_pha = bool(os.environ.get("KSKIP_PHA"))
    skip_at = bool(os.environ.get("KSKIP_ATTN"))
    skip_ga = bool(os.environ.get("KSKIP_GA"))
    skip_ar = bool(os.environ.get("KSKIP_AR"))
    skip_ld = bool(os.environ.get("KSKIP_LOADS"))
    nq_piece = CALLS_PER_PIECE * GC              # quads per store piece
    n_gpiece = nq_slots // nq_piece
    caps = {"v": caps_v, "c": caps_c}
    plan = {br: _pairs_from_caps(caps[br]) for br in ("v", "c")}
    ns = {br: plan[br][0] for br in ("v", "c")}
    pairs = {br: plan[br][1] for br in ("v", "c")}
    nchk1 = {br: ns[br] * SLOT // 128 for br in ("v", "c")}
    ngrp = {br: ns[br] // GRP for br in ("v", "c")}

    nc = bacc.Bacc("TRN2", num_swdge_queues=4)

    ins = {}
    for br in ("v", "c"):
        ins[f"sdat_{br}"] = nc.declare_dram_parameter(
            f"sdat_{br}", [128, nchk1[br], E], BF, isOutput=False)
        ins[f"crel2_{br}"] = nc.declare_dram_parameter(
            f"crel2_{br}", [128, len(pairs[br])], DT, isOutput=False)
        ins[f"invT_{br}"] = nc.declare_dram_parameter(
            f"invT_{br}", [E, VN], DT, isOutput=False)
        ins[f"gidx_{br}"] = nc.declare_dram_parameter(
            f"gidx_{br}", [128, nq_slots // 16], I16, isOutput=False)
        ins[f"semq_{br}"] = nc.declare_dram_parameter(
            f"semq_{br}", [E, QCH], BF, isOutput=False)
        ins[f"wqT_{br}"] = nc.declare_dram_parameter(f"wqT_{br}", [E, 128], BF, isOutput=False)
        ins[f"wkT_{br}"] = nc.declare_dram_parameter(f"wkT_{br}", [E, 128], BF, isOutput=False)
        ins[f"wvT_{br}"] = nc.declare_dram_parameter(f"wvT_{br}", [E, E], BF, isOutput=False)
        ins[f"woT_{br}"] = nc.declare_dram_parameter(f"woT_{br}", [E, E], BF, isOutput=False)
        ins[f"bq_{br}"] = nc.declare_dram_parameter(f"bq_{br}", [128, 1], DT, isOutput=False)
        ins[f"bk_{br}"] = nc.declare_dram_parameter(f"bk_{br}", [128, 1], DT, isOutput=False)
        ins[f"bv_{br}"] = nc.declare_dram_parameter(f"bv_{br}", [E, 1], DT, isOutput=False)
        ins[f"bo_{br}"] = nc.declare_dram_parameter(f"bo_{br}", [E, 1], DT, isOutput=False)
    ident = nc.declare_dram_parameter("ident", [128, 128], DT, isOutput=False)
    iotaF = nc.declare_dram_parameter("iotaF", [128, 128], BF, isOutput=False)
    bpat = nc.declare_dram_parameter("bpat", [128, SLOT_CH * 128], BF,
                                     isOutput=False)
    out_ext = nc.declare_dram_parameter(
        "out", [2, n_gpiece, 128, CALLS_PER_PIECE * (GC // 128), QELEM], BF,
        isOutput=True)

    acc = {br: nc.dram_tensor(f"acc_{br}", [E, VN], DT) for br in ("v", "c")}
    acc_red = {br: nc.dram_tensor(f"acc_red_{br}", [E, VN], DT,
                                  addr_space="Shared") for br in ("v", "c")}
    tbl_own = {br: nc.dram_tensor(f"tbl_own_{br}", [QCH, QELEM], BF)
               for br in ("v", "c")}
    tbl_all = {br: nc.dram_tensor(f"tbl_all_{br}", [N_CORES * QCH, QELEM], BF,
                                  addr_space="Shared") for br in ("v", "c")}

    rg = [list(range(N_CORES))]
    qn = [0]

    with tile.TileContext(nc) as tc:
        nc.gpsimd.load_library(library_config.mlp)

        cst_cm = tc.tile_pool(name="cst", bufs=1)
        cst = cst_cm.__enter__()
        iot = cst.tile([128, 128], BF, name="iot")
        nc.sync.dma_start(out=iot[:], in_=iotaF[:])
        idt = cst.tile([128, 128], DT, name="idt")
        nc.sync.dma_start(out=idt[:], in_=ident[:])
        bpt = cst.tile([128, SLOT_CH * 128], BF, name="bpt")
        nc.sync.dma_start(out=bpt[:], in_=bpat[:])
        bp3 = bpt[:].rearrange("p (j s) -> p j s", s=128)
        crel2 = {}
        for br in ("v", "c"):
            crel2[br] = cst.tile([128, len(pairs[br])], DT, name=f"crel2_{br}")
            nc.sync.dma_start(out=crel2[br][:], in_=ins[f"crel2_{br}"][:])

        def phase_a(br):
            """Two-stage PE segment sums -> acc[br] -> AllReduce."""
            sumsT = cst.tile([E, VN], DT, name=f"sumsT_{br}")
            if skip_pha:
                nc.vector.memset(sumsT[:], 0.0)
            else:
                sdat = ins[f"sdat_{br}"]
                NG = ngrp[br]
                GPP = 16                       # groups per load piece (1MB)
                npiece = (NG + GPP - 1) // GPP
                CPBLK = 4                      # groups per PSUM drain block
                ss = cst.tile([128, NG, E], BF, name=f"ss_{br}")
                with tc.tile_pool(name=f"ld_{br}", bufs=3) as ldp, \
                     tc.tile_pool(name=f"psA_{br}", bufs=2,
                                  space=bass.MemorySpace.PSUM) as psA:
                    for pi in range(npiece):
                        g0 = pi * GPP
                        g1 = min(NG, g0 + GPP)
                        nch = (g1 - g0) * SLOT_CH
                        if skip_ld:
                            if pi == 0:
                                pc = ldp.tile([128, GPP * SLOT_CH * E], BF,
                                              name="pc")
                                nc.vector.memset(pc[:], 0.0)
                        else:
                            pc = ldp.tile([128, GPP * SLOT_CH * E], BF,
                                          name="pc")
                            nc.sync.dma_start(
                                out=pc[:, 0:nch * E],
                                in_=sdat[:, g0 * SLOT_CH:g1 * SLOT_CH, :]
                                .rearrange("p c e -> p (c e)"))
                        pc3 = pc[:].rearrange("p (c e) -> p c e", e=E)
                        for b0 in range(g0, g1, CPBLK):
                            b1 = min(g1, b0 + CPBLK)
                            ps1 = psA.tile([128, CPBLK * E], DT, name="ps1")
                            p13 = ps1[:].rearrange("p (a e) -> p a e", e=E)
                            for g in range(b0, b1):
                                for j in range(SLOT_CH):
                                    cl = (g - g0) * SLOT_CH + j
                                    nc.tensor.matmul(
                                        p13[:, g - b0, :], bp3[:, j, :],
                                        pc3[:, cl, :],
                                        start=(j == 0), stop=(j == SLOT_CH - 1))
                            nc.scalar.activation(
                                ss[:, b0:b1, :].rearrange("p a e -> p (a e)"),
                                ps1[:, 0:(b1 - b0) * E],
                                mybir.ActivationFunctionType.Copy)
                # stage 2: slot sums -> class sums via is_equal one-hots
                with tc.tile_pool(name=f"oh2_{br}", bufs=4) as ohp, \
                     tc.tile_pool(name=f"ps2_{br}", bufs=2,
                                  space=bass.MemorySpace.PSUM) as ps2p:
                    pt = {}
                    for i, (g, w, first, last) in enumerate(pairs[br]):
                        oh = ohp.tile([128, 128], BF, name="oh2")
                        nc.vector.tensor_scalar(
                            oh[:], iot[:], crel2[br][:, i:i + 1],
                            None, mybir.AluOpType.is_equal)
                        if first:
                            pt[w] = ps2p.tile([E, 128], DT, name="pt2")
                        nc.tensor.matmul(
                            pt[w][:], ss[:, g, :], oh[:],
                            start=first, stop=last, skip_group_check=True)
                        if last:
                            nc.scalar.activation(
                                sumsT[:, w * 128:(w + 1) * 128], pt[w][:],
                                mybir.ActivationFunctionType.Copy)
            nc.sync.dma_start(out=acc[br][:], in_=sumsT[:])
            if skip_ar:
                nc.sync.dma_start(out=acc_red[br][:], in_=acc[br][:])
            else:
                nc.gpsimd.collective_compute(
                    "AllReduce", mybir.AluOpType.add,
                    ins=[acc[br][:]], outs=[acc_red[br][:]], replica_groups=rg)

        def attention(br):
            """acc_red[br] -> feaT -> attention -> tbl_own -> AllGather."""
            with tc.tile_pool(name=f"ap_{br}", bufs=1) as ap:
                feaF = ap.tile([E, VN], DT, name="feaF")
                nc.sync.dma_start(out=feaF[:], in_=acc_red[br][:])
                invT = ap.tile([E, VN], DT, name="invT")
                nc.sync.dma_start(out=invT[:], in_=ins[f"invT_{br}"][:])
                feaT = ap.tile([E, VN], BF, name="feaT")
                nc.vector.tensor_tensor(
                    feaT[:], feaF[:], invT[:], mybir.AluOpType.mult)

                wq = ap.tile([E, 128], BF, name="wq"); nc.sync.dma_start(out=wq[:], in_=ins[f"wqT_{br}"][:])
                wk = ap.tile([E, 128], BF, name="wk"); nc.sync.dma_start(out=wk[:], in_=ins[f"wkT_{br}"][:])
                wv = ap.tile([E, E], BF, name="wv"); nc.sync.dma_start(out=wv[:], in_=ins[f"wvT_{br}"][:])
                wo = ap.tile([E, E], BF, name="wo"); nc.sync.dma_start(out=wo[:], in_=ins[f"woT_{br}"][:])
                bq = ap.tile([128, 1], DT, name="bq"); nc.sync.dma_start(out=bq[:], in_=ins[f"bq_{br}"][:])
                bk = ap.tile([128, 1], DT, name="bk"); nc.sync.dma_start(out=bk[:], in_=ins[f"bk_{br}"][:])
                bv = ap.tile([E, 1], DT, name="bv"); nc.sync.dma_start(out=bv[:], in_=ins[f"bv_{br}"][:])
                bo = ap.tile([E, 1], DT, name="bo"); nc.sync.dma_start(out=bo[:], in_=ins[f"bo_{br}"][:])
                smq = ap.tile([E, QCH], BF, name="smq")
                nc.sync.dma_start(out=smq[:], in_=ins[f"semq_{br}"][:])

                ktile = ap.tile([128, VN], BF, name="ktile")
                qtile = ap.tile([128, QCH], BF, name="qtile")
                vtile = ap.tile([128, TC, 17 * H], BF, name="vtile")
                with tc.tile_pool(name=f"pP_{br}", bufs=2,
                                  space=bass.MemorySpace.PSUM) as pP:
                    for ch in range(VN // 512):
                        kps = pP.tile([128, 512], DT, name="kps")
                        nc.tensor.matmul(
                            kps[:], wk[:], feaT[:, ch * 512:(ch + 1) * 512])
                        nc.vector.tensor_scalar_add(
                            ktile[:, ch * 512:(ch + 1) * 512], kps[:], bk[:])
                    qps = pP.tile([128, QCH], DT, name="kps")
                    nc.tensor.matmul(qps[:], wq[:], smq[:])
                    nc.vector.tensor_scalar_add(qtile[:], qps[:], bq[:])

                    for h in range(H):
                        nc.vector.memset(vtile[:, :, 17 * h:17 * h + 1], 1.0)
                    for a in range(TC):
                        vps = pP.tile([128, E], DT, name="vps")
                        nc.tensor.matmul(
                            vps[:], feaT[:, a * 128:(a + 1) * 128], wv[:])
                        nc.vector.tensor_copy(
                            vtile[:, a, :].rearrange(
                                "p (h d) -> p h d", d=17)[:, :, 1:17],
                            vps[:].rearrange("p (h d) -> p h d", d=16))

                attnT = ap.tile([E, QCH], BF, name="attnT")
                with tc.tile_pool(name=f"pA_{br}", bufs=1,
                                  space=bass.MemorySpace.PSUM) as pA:
                    # all 4 heads in one PSUM bank, packed by col-group
                    av = pA.tile([128, QCH], DT, name="av")
                    with tc.tile_pool(name=f"pS_{br}", bufs=1,
                                      space=bass.MemorySpace.PSUM) as pS, \
                         tc.tile_pool(name=f"eS_{br}", bufs=2) as eS:
                        for a in ([0] if skip_at else range(TC)):
                            scf = pS.tile([128, H * QCH], DT, name="scf")
                            for h in range(H):
                                nc.tensor.matmul(
                                    scf[:, h * QCH:(h + 1) * QCH],
                                    ktile[32 * h:32 * h + 32,
                                          a * 128:(a + 1) * 128],
                                    qtile[32 * h:32 * h + 32, :],
                                    tile_position=(32 * h, 0))
                            exf = eS.tile([128, H * QCH], BF, name="exf")
                            nc.scalar.activation(
                                exf[:], scf[:],
                                mybir.ActivationFunctionType.Exp)
                            for h in range(H):
                                nc.tensor.matmul(
                                    av[32 * h:32 * h + 17, :],
                                    vtile[:, a, 17 * h:17 * h + 17],
                                    exf[:, h * QCH:(h + 1) * QCH],
                                    tile_position=(0, 32 * h),
                                    start=(a == 0),
                                    stop=(a == TC - 1 or skip_at),
                                    skip_group_check=True)

                    with tc.tile_pool(name=f"pN_{br}", bufs=1,
                                      space=bass.MemorySpace.PSUM) as pN, \
                         tc.tile_pool(name=f"eN_{br}", bufs=1) as eN:
                        one17 = eN.tile([1, 17], DT, name="one17")
                        nc.vector.memset(one17[:], 1.0)
                        for h in range(H):
                            rec = eN.tile([1, QCH], DT, name="rec", bufs=2)
                            nc.vector.reciprocal(rec[:], av[32 * h:32 * h + 1, :])
                            rbc = pN.tile([17, QCH], DT, name="rbc", bufs=2)
                            nc.tensor.matmul(rbc[:], one17[:], rec[:])
                            rbs = eN.tile([17, QCH], DT, name="rbs", bufs=2)
                            nc.vector.tensor_copy(rbs[:], rbc[:])
                            at_ = eN.tile([17, QCH], BF, name="at", bufs=2)
                            nc.vector.tensor_tensor(
                                at_[:], av[32 * h:32 * h + 17, :], rbs[:],
                                mybir.AluOpType.mult)
                            nc.sync.dma_start(
                                out=attnT[16 * h:16 * h + 16, :],
                                in_=at_[1:17, :])

                # + bv (softmax rows sum to 1), out-proj, transpose, quad-dup
                nc.vector.tensor_scalar_add(attnT[:], attnT[:], bv[:])
                dupt = ap.tile([128, QCH // 128, QELEM], BF, name="dupt")
                with tc.tile_pool(name=f"pF_{br}", bufs=2,
                                  space=bass.MemorySpace.PSUM) as pF:
                    fps = pF.tile([E, QCH], DT, name="fps", bufs=1)
                    nc.tensor.matmul(fps[:], wo[:], attnT[:])
                    fT = ap.tile([E, QCH], DT, name="fT")
                    nc.vector.tensor_scalar_add(fT[:], fps[:], bo[:])
                    for i in range(QCH // 128):
                        tp = pF.tile([128, E], DT, name="tp")
                        nc.tensor.transpose(
                            tp[:], fT[:, i * 128:(i + 1) * 128], idt[0:E, 0:E])
                        for d in range(4):
                            nc.vector.tensor_copy(
                                dupt[:, i, d * E:(d + 1) * E], tp[:])
                nc.sync.dma_start(
                    out=tbl_own[br][:].rearrange("(i p) d -> p i d", p=128),
                    in_=dupt[:])
            if skip_ar:
                nc.sync.dma_start(out=tbl_all[br][0:QCH, :], in_=tbl_own[br][:])
            else:
                nc.gpsimd.collective_compute(
                    "AllGather", mybir.AluOpType.bypass,
                    ins=[tbl_own[br][:]], outs=[tbl_all[br][:]],
                    replica_groups=rg)

        def gather(br):
            br_i = 0 if br == "v" else 1
            with tc.tile_pool(name=f"gp_{br}", bufs=3) as gp, \
                 tc.tile_pool(name=f"gip_{br}", bufs=1) as gip:
                git = gip.tile([128, nq_slots // 16], I16, name=f"git_{br}")
                nc.sync.dma_start(out=git[:], in_=ins[f"gidx_{br}"][:])
                for pi in range(n_gpiece):
                    gt = gp.tile(
                        [128, CALLS_PER_PIECE * (GC // 128) * QELEM], BF,
                        name="gt")
                    g3 = gt[:].rearrange("p (a f) -> p a f", f=QELEM)
                    if skip_ga:
                        nc.vector.memset(gt[:], 0.0)
                    else:
                        for ci in range(CALLS_PER_PIECE):
                            q0 = pi * CALLS_PER_PIECE * GC + ci * GC
                            nc.gpsimd.dma_gather(
                                g3[:, ci * (GC // 128):(ci + 1) * (GC // 128), :],
                                tbl_all[br][:],
                                git[:, q0 // 16:(q0 + GC) // 16],
                                GC, GC, QELEM,
                                queue_num=qn[0] % 4)
                            qn[0] += 1
                    nc.sync.dma_start(out=out_ext[br_i, pi], in_=g3)

        # staggered emission: v collective/attention/gather overlap c phase A
        phase_a("v")
        phase_a("c")
        attention("v")
        gather("v")
        attention("c")
        gather("c")
        cst_cm.__exit__(None, None, None)
    nc.compile()
    return nc


@functools.cache
def _compiled(nq_slots: int):
    return _build(nq_slots)


# ------------------------- host-side preparation -------------------------

def _wrap_idx(idx: np.ndarray) -> np.ndarray:
    n = idx.shape[0]
    w = np.ascontiguousarray(idx.reshape(n // 16, 16).T).astype(np.int16)
    return np.tile(w, (8, 1))


def _branch_weights(in_w, in_b, out_w, out_b):
    in_w = np.asarray(in_w, np.float32)
    in_b = np.asarray(in_b, np.float32)
    wq, wk, wv = in_w[:E], in_w[E:2 * E], in_w[2 * E:]
    bq, bk, bv = in_b[:E], in_b[E:2 * E], in_b[2 * E:]
    scale = np.float32(1.0 / np.sqrt(HD))
    wqT_pad = np.zeros((E, 128), np.float32)
    wkT_pad = np.zeros((E, 128), np.float32)
    bq_pad = np.zeros((128, 1), np.float32)
    bk_pad = np.zeros((128, 1), np.float32)
    for h in range(H):
        for j in range(HD):
            wqT_pad[:, 32 * h + j] = wq[HD * h + j] * scale
            wkT_pad[:, 32 * h + j] = wk[HD * h + j]
            bq_pad[32 * h + j, 0] = bq[HD * h + j] * scale
            bk_pad[32 * h + j, 0] = bk[HD * h + j]
    return {
        "wqT": wqT_pad.astype(BF_NP), "wkT": wkT_pad.astype(BF_NP),
        "wvT": np.ascontiguousarray(wv.T).astype(BF_NP),
        "woT": np.ascontiguousarray(
            np.asarray(out_w, np.float32).T).astype(BF_NP),
        "bq": bq_pad, "bk": bk_pad,
        "bv": bv.reshape(E, 1).astype(np.float32),
        "bo": np.asarray(out_b, np.float32).reshape(E, 1),
    }


def _prep_phase_a(s_bf: np.ndarray, cls: np.ndarray):
    """Sort shard rows by class, deal into 32 windows x 33 chunks x 128 rows.
    Returns (sdat [128,NCHK,E] bf16, crel [128,NCHK] f32)."""
    n = cls.shape[0]
    order = np.argsort(cls, kind="stable")
    scls = cls[order].astype(np.int64)
    win = scls >> 7
    wstart = np.searchsorted(win, np.arange(WIN))
    wcount = np.diff(np.r_[wstart, n])
    if wcount.max() > WCAP * 128:
        raise RuntimeError(f"window overflow: {wcount.max()} > {WCAP * 128}")
    rank = np.arange(n) - wstart[win]
    slot = win * (WCAP * 128) + rank
    rows = np.zeros((NSLOT, E), BF_NP)
    rows[slot] = s_bf[order]
    crel_f = np.zeros(NSLOT, np.float32)
    crel_f[slot] = (scls & 127).astype(np.float32)
    sdat = np.ascontiguousarray(rows.reshape(NCHK, 128, E).transpose(1, 0, 2))
    crel = np.ascontiguousarray(crel_f.reshape(NCHK, 128).T)
    return sdat, crel


def _prep_quads(cls: np.ndarray, nq_slots: int):
    """Group same-class rows (sorted order) into quads.
    Returns (gidx [128, nq_slots/16] int16, q_global [n], qslot [n])."""
    n = cls.shape[0]
    order = np.argsort(cls, kind="stable")
    scls = cls[order].astype(np.int64)
    cnt = np.bincount(cls, minlength=VN)
    cstart = np.r_[0, np.cumsum(cnt)]
    rank = np.arange(n) - cstart[scls]
    nquad = (cnt + 3) // 4
    qbase = np.r_[0, np.cumsum(nquad)]
    q_of_sorted = qbase[scls] + (rank >> 2)
    nq_real = int(qbase[-1])
    assert nq_real <= nq_slots, (nq_real, nq_slots)
    qcls = np.zeros(nq_slots, np.int64)
    qcls[q_of_sorted] = scls
    # AllGather table layout: row == class id
    q_global = np.empty(n, np.int64)
    q_global[order] = q_of_sorted
    qslot = np.empty(n, np.int64)
    qslot[order] = rank & 3
    return _wrap_idx(qcls), q_global, qslot


def _make_plan(v_class, c_class):
    """nq_slots: max quads over cores/branches, rounded to a piece."""
    nq_max = 0
    for cls_all in (v_class, c_class):
        for core in range(N_CORES):
            cls = cls_all[core * SHARD:(core + 1) * SHARD]
            cnt = np.bincount(cls, minlength=VN)
            nq_max = max(nq_max, int(((cnt + 3) // 4).sum()))
    piece = CALLS_PER_PIECE * GC
    return ((nq_max + piece - 1) // piece) * piece


def _make_in_maps(v_s, c_s, v_sem, c_sem, v_class, c_class,
                  v_in_w, v_in_b, v_out_w, v_out_b,
                  c_in_w, c_in_b, c_out_w, c_out_b, nq_slots):
    v_class = np.asarray(v_class, np.int32)
    c_class = np.asarray(c_class, np.int32)
    v_bf = np.asarray(v_s, np.float32).astype(BF_NP)
    c_bf = np.asarray(c_s, np.float32).astype(BF_NP)
    v_semT = np.ascontiguousarray(np.asarray(v_sem, np.float32).T)
    c_semT = np.ascontiguousarray(np.asarray(c_sem, np.float32).T)
    wts = {"v": _branch_weights(v_in_w, v_in_b, v_out_w, v_out_b),
           "c": _branch_weights(c_in_w, c_in_b, c_out_w, c_out_b)}
    ident = np.eye(128, dtype=np.float32)
    iotaF = np.ascontiguousarray(
        np.broadcast_to(np.arange(128, dtype=np.float32), (128, 128))
    ).astype(BF_NP)
    invT = {}
    for br, cls in (("v", v_class), ("c", c_class)):
        cnt = np.bincount(cls, minlength=VN).astype(np.float32)
        inv = (1.0 / (cnt + 1e-8)).astype(np.float32)
        invT[br] = np.ascontiguousarray(
            np.broadcast_to(inv[None, :], (E, VN)))
    in_maps = []
    unmaps = []
    for core in range(N_CORES):
        b0 = core * SHARD
        m = {"ident": ident, "iotaF": iotaF}
        um = {}
        for br, s_bf, cls_all, semT in (
                ("v", v_bf, v_class, v_semT), ("c", c_bf, c_class, c_semT)):
            cls = cls_all[b0:b0 + SHARD]
            sdat, crel = _prep_phase_a(s_bf[b0:b0 + SHARD], cls)
            gidx, q_global, qslot = _prep_quads(cls, nq_slots)
            m[f"sdat_{br}"] = sdat
            m[f"crel_{br}"] = crel
            m[f"gidx_{br}"] = gidx
            m[f"invT_{br}"] = invT[br]
            m[f"semq_{br}"] = np.ascontiguousarray(
                semT[:, core * QCH:(core + 1) * QCH]).astype(BF_NP)
            for k, vv in wts[br].items():
                m[f"{k}_{br}"] = vv
            um[br] = (q_global, qslot)
        in_maps.append(m)
        unmaps.append(um)
    return in_maps, unmaps


def _unpack_out(res_results, unmaps, nq_slots):
    """out_ext [2, n_gpiece, 128, CP*8, QELEM] bf16 -> full f32 outputs."""
    n_gpiece = nq_slots // (CALLS_PER_PIECE * GC)
    v_out = np.empty((NV, E), np.float32)
    c_out = np.empty((NV, E), np.float32)
    for core in range(N_CORES):
        o = np.asarray(res_results[core]["out"])
        if o.dtype != BF_NP:
            o = o.view(BF_NP)
        o = o.reshape(2, n_gpiece, 128, CALLS_PER_PIECE, GC // 128, QELEM)
        # quad q = piece*CP*GC + ci*GC + jj*128 + p  at o[br, piece, p, ci, jj]
        quads = np.ascontiguousarray(
            o.transpose(0, 1, 3, 4, 2, 5)).reshape(
                2, nq_slots, 4, E).astype(np.float32)
        for br_i, (br, out) in enumerate((("v", v_out), ("c", c_out))):
            q_global, qslot = unmaps[core][br]
            out[core * SHARD:(core + 1) * SHARD] = \
                quads[br_i, q_global, qslot, :]
    return v_out, c_out


def kernel(v_s, c_s, v_sem, c_sem, v_class, c_class,
           v_in_w, v_in_b, v_out_w, v_out_b,
           c_in_w, c_in_b, c_out_w, c_out_b):
    nq_slots = _make_plan(np.asarray(v_class, np.int32),
                          np.asarray(c_class, np.int32))
    in_maps, unmaps = _make_in_maps(
        v_s, c_s, v_sem, c_sem, v_class, c_class,
        v_in_w, v_in_b, v_out_w, v_out_b,
        c_in_w, c_in_b, c_out_w, c_out_b, nq_slots)
    nc = _compiled(nq_slots)
    res = run_bass_kernel_spmd(nc, in_maps, core_ids=list(range(N_CORES)))
    return _unpack_out(res.results, unmaps, nq_slots)


# exposed for test.py timing
def prepare_in_maps(inputs):
    sig = ["v_s", "c_s", "v_sem", "c_sem", "v_class", "c_class",
           "v_in_w", "v_in_b", "v_out_w", "v_out_b",
           "c_in_w", "c_in_b", "c_out_w", "c_out_b"]
    kw = {k: inputs[k] for k in sig}
    nq_slots = _make_plan(np.asarray(kw["v_class"], np.int32),
                          np.asarray(kw["c_class"], np.int32))
    in_maps, _ = _make_in_maps(**kw, nq_slots=nq_slots)
    return _compiled(nq_slots), in_maps


# revision 11
# speedup vs baseline: 1.3304x; 1.3304x over previous
"""Trainium2 Bass kernel for nn_Anchor3 (segment_reduce): 8-core SPMD, v3.

Per core (125k nodes/branch, bf16 data path):
  - segment-sum on the TensorEngine: host sorts each shard by class and
    deals rows into 32 windows of 128 classes (33 chunks of 128 rows per
    window, zero-padded); per chunk the DVE builds a one-hot [row, cls]
    via is_equal(iota, cls_rel) and the PE contracts rows:
    psumT[64, 128] += sdat_chunk[128, 64].T @ onehot[128, 128]
  - per-branch AllReduce of [64, 4096] f32 partial sums (v's collective and
    attention overlap c's phase A); multiply by host 1/(cnt+eps) -> feaT bf16
  - class-level cross-attention in bf16 (queries sharded 512/core): per-head
    padded weights, 4 heads as row-group matmuls for scores and col-group
    matmuls for attn@V (all concurrent via tile_position), single Exp per key
    chunk (f32 in PSUM -> bf16 out), fused sum-of-exp column, division
    deferred past the V-contraction
  - attention output rows are cast to bf16 and quad-replicated into a
    [512, 256] table slice; per-branch AllGather -> [4096, 256] table
  - output gather: host groups same-class rows into quads; SWDGE dma_gather
    (elem=512B, 1024 idx/call, 4 queues) pulls one 4-row quad per index;
    bulk stores write [128, 32, 256] bf16 pieces; host un-sorts.
Host does index-metadata prep only (sort, counts, quad layout) plus the
bf16 casts that define the kernel's working precision.
"""
import functools
import os

import numpy as np
import ml_dtypes

import concourse.bass as bass
import concourse.bacc as bacc
import concourse.mybir as mybir
import concourse.tile as tile
from concourse import library_config
from concourse.bass_utils import run_bass_kernel_spmd

N_CORES = 8
NV = 1_000_000
VN = 4096          # classes per branch
E = 64
H = 4
HD = 16
SHARD = NV // N_CORES            # 125000

WIN = 32                         # class windows per branch (128 classes each)
SLOT = 4                         # rows per slot (class-padded)
GRP = 128                        # slots per stage-1 group (= 512 rows)
SLOT_CH = GRP * SLOT // 128      # stage-1 chunks per group = 4

GC = 1024                        # gather idx per call (SWDGE ring-safe)
QELEM = 256                      # bf16 elems per gathered quad (512B)
CALLS_PER_PIECE = 4              # 4 gather calls -> one 2MB store
QCH = VN // N_CORES              # 512 query rows per core
TC = VN // 128                   # 32 key chunks in attention

DT = mybir.dt.float32
BF = mybir.dt.bfloat16
I16 = mybir.dt.int16
BF_NP = ml_dtypes.bfloat16


def _pairs_from_caps(caps):
    """Static stage-2 pair list from per-window slot capacities.
    Returns (ns, [(g, w, first, last), ...]) ordered by g then w."""
    off = np.r_[0, np.cumsum(caps)]
    ns_used = int(off[-1])
    ns = ((ns_used + GRP - 1) // GRP) * GRP
    pairs = []
    for g in range(ns // GRP):
        lo, hi = g * GRP, (g + 1) * GRP
        for w in range(len(caps)):
            if off[w] < hi and off[w + 1] > lo:
                pairs.append([g, w])
    firsts, lasts = {}, {}
    for i, (g, w) in enumerate(pairs):
        firsts.setdefault(w, i)
        lasts[w] = i
    return ns, [(g, w, firsts[w] == i, lasts[w] == i)
                for i, (g, w) in enumerate(pairs)]


def _build(nq_slots: int, caps_v: tuple, caps_c: tuple):
    """nq_slots: gather quads per branch (multiple of CALLS_PER_PIECE*GC)."""
    skip_pha = bool(os.environ.get("KSKIP_PHA"))
    skip_at = bool(os.environ.get("KSKIP_ATTN"))
    skip_ga = bool(os.environ.get("KSKIP_GA"))
    skip_ar = bool(os.environ.get("KSKIP_AR"))
    skip_ld = bool(os.environ.get("KSKIP_LOADS"))
    nq_piece = CALLS_PER_PIECE * GC              # quads per store piece
    n_gpiece = nq_slots // nq_piece
    caps = {"v": caps_v, "c": caps_c}
    plan = {br: _pairs_from_caps(caps[br]) for br in ("v", "c")}
    ns = {br: plan[br][0] for br in ("v", "c")}
    pairs = {br: plan[br][1] for br in ("v", "c")}
    nchk1 = {br: ns[br] * SLOT // 128 for br in ("v", "c")}
    ngrp = {br: ns[br] // GRP for br in ("v", "c")}

    nc = bacc.Bacc("TRN2", num_swdge_queues=4)

    ins = {}
    for br in ("v", "c"):
        ins[f"sdat_{br}"] = nc.declare_dram_parameter(
            f"sdat_{br}", [128, nchk1[br], E], BF, isOutput=False)
        ins[f"crel2_{br}"] = nc.declare_dram_parameter(
            f"crel2_{br}", [128, len(pairs[br])], DT, isOutput=False)
        ins[f"invT_{br}"] = nc.declare_dram_parameter(
            f"invT_{br}", [E, VN], DT, isOutput=False)
        ins[f"gidx_{br}"] = nc.declare_dram_parameter(
            f"gidx_{br}", [128, nq_slots // 16], I16, isOutput=False)
        ins[f"semq_{br}"] = nc.declare_dram_parameter(
            f"semq_{br}", [E, QCH], BF, isOutput=False)
        ins[f"wqT_{br}"] = nc.declare_dram_parameter(f"wqT_{br}", [E, 128], BF, isOutput=False)
        ins[f"wkT_{br}"] = nc.declare_dram_parameter(f"wkT_{br}", [E, 128], BF, isOutput=False)
        ins[f"wvT_{br}"] = nc.declare_dram_parameter(f"wvT_{br}", [E, E], BF, isOutput=False)
        ins[f"woT_{br}"] = nc.declare_dram_parameter(f"woT_{br}", [E, E], BF, isOutput=False)
        ins[f"bq_{br}"] = nc.declare_dram_parameter(f"bq_{br}", [128, 1], DT, isOutput=False)
        ins[f"bk_{br}"] = nc.declare_dram_parameter(f"bk_{br}", [128, 1], DT, isOutput=False)
        ins[f"bv_{br}"] = nc.declare_dram_parameter(f"bv_{br}", [E, 1], DT, isOutput=False)
        ins[f"bo_{br}"] = nc.declare_dram_parameter(f"bo_{br}", [E, 1], DT, isOutput=False)
    ident = nc.declare_dram_parameter("ident", [128, 128], DT, isOutput=False)
    iotaF = nc.declare_dram_parameter("iotaF", [128, 128], BF, isOutput=False)
    bpat = nc.declare_dram_parameter("bpat", [128, SLOT_CH * 128], BF,
                                     isOutput=False)
    out_ext = nc.declare_dram_parameter(
        "out", [2, n_gpiece, 128, CALLS_PER_PIECE * (GC // 128), QELEM], BF,
        isOutput=True)

    acc = {br: nc.dram_tensor(f"acc_{br}", [E, VN], DT) for br in ("v", "c")}
    acc_red = {br: nc.dram_tensor(f"acc_red_{br}", [E, VN], DT,
                                  addr_space="Shared") for br in ("v", "c")}
    tbl_own = {br: nc.dram_tensor(f"tbl_own_{br}", [QCH, QELEM], BF)
               for br in ("v", "c")}
    tbl_all = {br: nc.dram_tensor(f"tbl_all_{br}", [N_CORES * QCH, QELEM], BF,
                                  addr_space="Shared") for br in ("v", "c")}

    rg = [list(range(N_CORES))]
    qn = [0]

    with tile.TileContext(nc) as tc:
        nc.gpsimd.load_library(library_config.mlp)

        cst_cm = tc.tile_pool(name="cst", bufs=1)
        cst = cst_cm.__enter__()
        iot = cst.tile([128, 128], BF, name="iot")
        nc.sync.dma_start(out=iot[:], in_=iotaF[:])
        idt = cst.tile([128, 128], DT, name="idt")
        nc.sync.dma_start(out=idt[:], in_=ident[:])
        bpt = cst.tile([128, SLOT_CH * 128], BF, name="bpt")
        nc.sync.dma_start(out=bpt[:], in_=bpat[:])
        bp3 = bpt[:].rearrange("p (j s) -> p j s", s=128)
        crel2 = {}
        for br in ("v", "c"):
            crel2[br] = cst.tile([128, len(pairs[br])], DT, name=f"crel2_{br}")
            nc.sync.dma_start(out=crel2[br][:], in_=ins[f"crel2_{br}"][:])

        def phase_a(br):
            """Two-stage PE segment sums -> acc[br] -> AllReduce."""
            sumsT = cst.tile([E, VN], DT, name=f"sumsT_{br}")
            if skip_pha:
                nc.vector.memset(sumsT[:], 0.0)
            else:
                sdat = ins[f"sdat_{br}"]
                NG = ngrp[br]
                GPP = 16                       # groups per load piece (1MB)
                npiece = (NG + GPP - 1) // GPP
                CPBLK = 4                      # groups per PSUM drain block
                ss = cst.tile([128, NG, E], BF, name=f"ss_{br}")
                with tc.tile_pool(name=f"ld_{br}", bufs=3) as ldp, \
                     tc.tile_pool(name=f"psA_{br}", bufs=2,
                                  space=bass.MemorySpace.PSUM) as psA:
                    for pi in range(npiece):
                        g0 = pi * GPP
                        g1 = min(NG, g0 + GPP)
                        nch = (g1 - g0) * SLOT_CH
                        if skip_ld:
                            if pi == 0:
                                pc = ldp.tile([128, GPP * SLOT_CH * E], BF,
                                              name="pc")
                                nc.vector.memset(pc[:], 0.0)
                        else:
                            pc = ldp.tile([128, GPP * SLOT_CH * E], BF,
                                          name="pc")
                            nc.sync.dma_start(
                                out=pc[:, 0:nch * E],
                                in_=sdat[:, g0 * SLOT_CH:g1 * SLOT_CH, :]
                                .rearrange("p c e -> p (c e)"))
                        pc3 = pc[:].rearrange("p (c e) -> p c e", e=E)
                        for b0 in range(g0, g1, CPBLK):
                            b1 = min(g1, b0 + CPBLK)
                            ps1 = psA.tile([128, CPBLK * E], DT, name="ps1")
                            p13 = ps1[:].rearrange("p (a e) -> p a e", e=E)
                            for g in range(b0, b1):
                                for j in range(SLOT_CH):
                                    cl = (g - g0) * SLOT_CH + j
                                    nc.tensor.matmul(
                                        p13[:, g - b0, :], bp3[:, j, :],
                                        pc3[:, cl, :],
                                        start=(j == 0), stop=(j == SLOT_CH - 1))
                            nc.scalar.activation(
                                ss[:, b0:b1, :].rearrange("p a e -> p (a e)"),
                                ps1[:, 0:(b1 - b0) * E],
                                mybir.ActivationFunctionType.Copy)
                # stage 2: slot sums -> class sums via is_equal one-hots
                with tc.tile_pool(name=f"oh2_{br}", bufs=4) as ohp, \
                     tc.tile_pool(name=f"ps2_{br}", bufs=2,
                                  space=bass.MemorySpace.PSUM) as ps2p:
                    pt = {}
                    for i, (g, w, first, last) in enumerate(pairs[br]):
                        oh = ohp.tile([128, 128], BF, name="oh2")
                        nc.vector.tensor_scalar(
                            oh[:], iot[:], crel2[br][:, i:i + 1],
                            None, mybir.AluOpType.is_equal)
                        if first:
                            pt[w] = ps2p.tile([E, 128], DT, name="pt2")
                        nc.tensor.matmul(
                            pt[w][:], ss[:, g, :], oh[:],
                            start=first, stop=last, skip_group_check=True)
                        if last:
                            nc.scalar.activation(
                                sumsT[:, w * 128:(w + 1) * 128], pt[w][:],
                                mybir.ActivationFunctionType.Copy)
            nc.sync.dma_start(out=acc[br][:], in_=sumsT[:])
            if skip_ar:
                nc.sync.dma_start(out=acc_red[br][:], in_=acc[br][:])
            else:
                nc.gpsimd.collective_compute(
                    "AllReduce", mybir.AluOpType.add,
                    ins=[acc[br][:]], outs=[acc_red[br][:]], replica_groups=rg)

        def attention(br):
            """acc_red[br] -> feaT -> attention -> tbl_own -> AllGather."""
            with tc.tile_pool(name=f"ap_{br}", bufs=1) as ap:
                feaF = ap.tile([E, VN], DT, name="feaF")
                nc.sync.dma_start(out=feaF[:], in_=acc_red[br][:])
                invT = ap.tile([E, VN], DT, name="invT")
                nc.sync.dma_start(out=invT[:], in_=ins[f"invT_{br}"][:])
                feaT = ap.tile([E, VN], BF, name="feaT")
                nc.vector.tensor_tensor(
                    feaT[:], feaF[:], invT[:], mybir.AluOpType.mult)

                wq = ap.tile([E, 128], BF, name="wq"); nc.sync.dma_start(out=wq[:], in_=ins[f"wqT_{br}"][:])
                wk = ap.tile([E, 128], BF, name="wk"); nc.sync.dma_start(out=wk[:], in_=ins[f"wkT_{br}"][:])
                wv = ap.tile([E, E], BF, name="wv"); nc.sync.dma_start(out=wv[:], in_=ins[f"wvT_{br}"][:])
                wo = ap.tile([E, E], BF, name="wo"); nc.sync.dma_start(out=wo[:], in_=ins[f"woT_{br}"][:])
                bq = ap.tile([128, 1], DT, name="bq"); nc.sync.dma_start(out=bq[:], in_=ins[f"bq_{br}"][:])
                bk = ap.tile([128, 1], DT, name="bk"); nc.sync.dma_start(out=bk[:], in_=ins[f"bk_{br}"][:])
                bv = ap.tile([E, 1], DT, name="bv"); nc.sync.dma_start(out=bv[:], in_=ins[f"bv_{br}"][:])
                bo = ap.tile([E, 1], DT, name="bo"); nc.sync.dma_start(out=bo[:], in_=ins[f"bo_{br}"][:])
                smq = ap.tile([E, QCH], BF, name="smq")
                nc.sync.dma_start(out=smq[:], in_=ins[f"semq_{br}"][:])

                ktile = ap.tile([128, VN], BF, name="ktile")
                qtile = ap.tile([128, QCH], BF, name="qtile")
                vtile = ap.tile([128, TC, 17 * H], BF, name="vtile")
                with tc.tile_pool(name=f"pP_{br}", bufs=2,
                                  space=bass.MemorySpace.PSUM) as pP:
                    for ch in range(VN // 512):
                        kps = pP.tile([128, 512], DT, name="kps")
                        nc.tensor.matmul(
                            kps[:], wk[:], feaT[:, ch * 512:(ch + 1) * 512])
                        nc.vector.tensor_scalar_add(
                            ktile[:, ch * 512:(ch + 1) * 512], kps[:], bk[:])
                    qps = pP.tile([128, QCH], DT, name="kps")
                    nc.tensor.matmul(qps[:], wq[:], smq[:])
                    nc.vector.tensor_scalar_add(qtile[:], qps[:], bq[:])

                    for h in range(H):
                        nc.vector.memset(vtile[:, :, 17 * h:17 * h + 1], 1.0)
                    for a in range(TC):
                        vps = pP.tile([128, E], DT, name="vps")
                        nc.tensor.matmul(
                            vps[:], feaT[:, a * 128:(a + 1) * 128], wv[:])
                        nc.vector.tensor_copy(
                            vtile[:, a, :].rearrange(
                                "p (h d) -> p h d", d=17)[:, :, 1:17],
                            vps[:].rearrange("p (h d) -> p h d", d=16))

                attnT = ap.tile([E, QCH], BF, name="attnT")
                with tc.tile_pool(name=f"pA_{br}", bufs=1,
                                  space=bass.MemorySpace.PSUM) as pA:
                    # all 4 heads in one PSUM bank, packed by col-group
                    av = pA.tile([128, QCH], DT, name="av")
                    with tc.tile_pool(name=f"pS_{br}", bufs=1,
                                      space=bass.MemorySpace.PSUM) as pS, \
                         tc.tile_pool(name=f"eS_{br}", bufs=2) as eS:
                        for a in ([0] if skip_at else range(TC)):
                            scf = pS.tile([128, H * QCH], DT, name="scf")
                            for h in range(H):
                                nc.tensor.matmul(
                                    scf[:, h * QCH:(h + 1) * QCH],
                                    ktile[32 * h:32 * h + 32,
                                          a * 128:(a + 1) * 128],
                                    qtile[32 * h:32 * h + 32, :],
                                    tile_position=(32 * h, 0))
                            exf = eS.tile([128, H * QCH], BF, name="exf")
                            nc.scalar.activation(
                                exf[:], scf[:],
                                mybir.ActivationFunctionType.Exp)
                            for h in range(H):
                                nc.tensor.matmul(
                                    av[32 * h:32 * h + 17, :],
                                    vtile[:, a, 17 * h:17 * h + 17],
                                    exf[:, h * QCH:(h + 1) * QCH],
                                    tile_position=(0, 32 * h),
                                    start=(a == 0),
                                    stop=(a == TC - 1 or skip_at),
                                    skip_group_check=True)

                    with tc.tile_pool(name=f"pN_{br}", bufs=1,
                                      space=bass.MemorySpace.PSUM) as pN, \
                         tc.tile_pool(name=f"eN_{br}", bufs=1) as eN:
                        one17 = eN.tile([1, 17], DT, name="one17")
                        nc.vector.memset(one17[:], 1.0)
                        for h in range(H):
                            rec = eN.tile([1, QCH], DT, name="rec", bufs=2)
                            nc.vector.reciprocal(rec[:], av[32 * h:32 * h + 1, :])
                            rbc = pN.tile([17, QCH], DT, name="rbc", bufs=2)
                            nc.tensor.matmul(rbc[:], one17[:], rec[:])
                            rbs = eN.tile([17, QCH], DT, name="rbs", bufs=2)
                            nc.vector.tensor_copy(rbs[:], rbc[:])
                            at_ = eN.tile([17, QCH], BF, name="at", bufs=2)
                            nc.vector.tensor_tensor(
                                at_[:], av[32 * h:32 * h + 17, :], rbs[:],
                                mybir.AluOpType.mult)
                            nc.sync.dma_start(
                                out=attnT[16 * h:16 * h + 16, :],
                                in_=at_[1:17, :])

                # + bv (softmax rows sum to 1), out-proj, transpose, quad-dup
                nc.vector.tensor_scalar_add(attnT[:], attnT[:], bv[:])
                dupt = ap.tile([128, QCH // 128, QELEM], BF, name="dupt")
                with tc.tile_pool(name=f"pF_{br}", bufs=2,
                                  space=bass.MemorySpace.PSUM) as pF:
                    fps = pF.tile([E, QCH], DT, name="fps", bufs=1)
                    nc.tensor.matmul(fps[:], wo[:], attnT[:])
                    fT = ap.tile([E, QCH], DT, name="fT")
                    nc.vector.tensor_scalar_add(fT[:], fps[:], bo[:])
                    for i in range(QCH // 128):
                        tp = pF.tile([128, E], DT, name="tp")
                        nc.tensor.transpose(
                            tp[:], fT[:, i * 128:(i + 1) * 128], idt[0:E, 0:E])
                        for d in range(4):
                            nc.vector.tensor_copy(
                                dupt[:, i, d * E:(d + 1) * E], tp[:])
                nc.sync.dma_start(
                    out=tbl_own[br][:].rearrange("(i p) d -> p i d", p=128),
                    in_=dupt[:])
            if skip_ar:
                nc.sync.dma_start(out=tbl_all[br][0:QCH, :], in_=tbl_own[br][:])
            else:
                nc.gpsimd.collective_compute(
                    "AllGather", mybir.AluOpType.bypass,
                    ins=[tbl_own[br][:]], outs=[tbl_all[br][:]],
                    replica_groups=rg)

        def gather(br):
            br_i = 0 if br == "v" else 1
            with tc.tile_pool(name=f"gp_{br}", bufs=3) as gp, \
                 tc.tile_pool(name=f"gip_{br}", bufs=1) as gip:
                git = gip.tile([128, nq_slots // 16], I16, name=f"git_{br}")
                nc.sync.dma_start(out=git[:], in_=ins[f"gidx_{br}"][:])
                for pi in range(n_gpiece):
                    gt = gp.tile(
                        [128, CALLS_PER_PIECE * (GC // 128) * QELEM], BF,
                        name="gt")
                    g3 = gt[:].rearrange("p (a f) -> p a f", f=QELEM)
                    if skip_ga:
                        nc.vector.memset(gt[:], 0.0)
                    else:
                        for ci in range(CALLS_PER_PIECE):
                            q0 = pi * CALLS_PER_PIECE * GC + ci * GC
                            nc.gpsimd.dma_gather(
                                g3[:, ci * (GC // 128):(ci + 1) * (GC // 128), :],
                                tbl_all[br][:],
                                git[:, q0 // 16:(q0 + GC) // 16],
                                GC, GC, QELEM,
                                queue_num=qn[0] % 4)
                            qn[0] += 1
                    nc.sync.dma_start(out=out_ext[br_i, pi], in_=g3)

        # staggered emission: v collective/attention/gather overlap c phase A
        phase_a("v")
        phase_a("c")
        attention("v")
        gather("v")
        attention("c")
        gather("c")
        cst_cm.__exit__(None, None, None)
    nc.compile()
    return nc


@functools.cache
def _compiled(nq_slots: int, caps_v: tuple, caps_c: tuple):
    return _build(nq_slots, caps_v, caps_c)


# ------------------------- host-side preparation -------------------------

def _wrap_idx(idx: np.ndarray) -> np.ndarray:
    n = idx.shape[0]
    w = np.ascontiguousarray(idx.reshape(n // 16, 16).T).astype(np.int16)
    return np.tile(w, (8, 1))


def _branch_weights(in_w, in_b, out_w, out_b):
    in_w = np.asarray(in_w, np.float32)
    in_b = np.asarray(in_b, np.float32)
    wq, wk, wv = in_w[:E], in_w[E:2 * E], in_w[2 * E:]
    bq, bk, bv = in_b[:E], in_b[E:2 * E], in_b[2 * E:]
    scale = np.float32(1.0 / np.sqrt(HD))
    wqT_pad = np.zeros((E, 128), np.float32)
    wkT_pad = np.zeros((E, 128), np.float32)
    bq_pad = np.zeros((128, 1), np.float32)
    bk_pad = np.zeros((128, 1), np.float32)
    for h in range(H):
        for j in range(HD):
            wqT_pad[:, 32 * h + j] = wq[HD * h + j] * scale
            wkT_pad[:, 32 * h + j] = wk[HD * h + j]
            bq_pad[32 * h + j, 0] = bq[HD * h + j] * scale
            bk_pad[32 * h + j, 0] = bk[HD * h + j]
    return {
        "wqT": wqT_pad.astype(BF_NP), "wkT": wkT_pad.astype(BF_NP),
        "wvT": np.ascontiguousarray(wv.T).astype(BF_NP),
        "woT": np.ascontiguousarray(
            np.asarray(out_w, np.float32).T).astype(BF_NP),
        "bq": bq_pad, "bk": bk_pad,
        "bv": bv.reshape(E, 1).astype(np.float32),
        "bo": np.asarray(out_b, np.float32).reshape(E, 1),
    }


def _slot_caps(cls: np.ndarray):
    """Per-window slot counts for one shard: [32] ints."""
    cnt = np.bincount(cls, minlength=VN)
    slots_per_class = (cnt + SLOT - 1) // SLOT
    return slots_per_class.reshape(WIN, 128).sum(axis=1)


def _prep_phase_a(s_bf: np.ndarray, cls: np.ndarray, caps):
    """Deal sorted rows into per-class slots of SLOT rows, windows padded to
    caps. Returns (sdat [128,nchk1,E] bf16, cls2 [ns] int64 class of slot,
    ns)."""
    n = cls.shape[0]
    ns, _ = _pairs_from_caps(caps)
    off = np.r_[0, np.cumsum(caps)]
    order = np.argsort(cls, kind="stable")
    scls = cls[order].astype(np.int64)
    cnt = np.bincount(cls, minlength=VN).astype(np.int64)
    cstart = np.r_[0, np.cumsum(cnt)]
    rank = np.arange(n) - cstart[scls]
    spc = (cnt + SLOT - 1) // SLOT            # slots per class
    # within-window slot base of each class
    win_of_class = np.arange(VN) >> 7
    sbase_in_win = np.zeros(VN, np.int64)
    for w in range(WIN):
        sel = slice(w * 128, (w + 1) * 128)
        sbase_in_win[sel] = np.r_[0, np.cumsum(spc[sel])[:-1]]
        used = spc[sel].sum()
        assert used <= caps[w], (w, used, caps[w])
    slot_of_class = off[win_of_class] + sbase_in_win   # global first slot
    slot = slot_of_class[scls] + (rank >> 2)
    pos = slot * SLOT + (rank & 3)
    nrow = ns * SLOT
    rows = np.zeros((nrow, E), BF_NP)
    rows[pos] = s_bf[order]
    cls2 = np.full(ns, -1, np.int64)
    cls2[slot] = scls
    sdat = np.ascontiguousarray(
        rows.reshape(nrow // 128, 128, E).transpose(1, 0, 2))
    return sdat, cls2, ns


def _prep_crel2(cls2: np.ndarray, caps):
    """crel2 [128, npairs] f32 for the static pair list."""
    ns, pairs = _pairs_from_caps(caps)
    out = np.full((128, len(pairs)), -1.0, np.float32)
    for i, (g, w, _f, _l) in enumerate(pairs):
        sl = cls2[g * GRP:(g + 1) * GRP]
        rel = sl - 128 * w
        ok = (sl >= 0) & (rel >= 0) & (rel < 128)
        col = np.full(128, -1.0, np.float32)
        col[ok] = rel[ok].astype(np.float32)
        out[:, i] = col
    return out


def _blockpat():
    bp = np.zeros((128, SLOT_CH, 128), np.float32)
    r = np.arange(128)
    for j in range(SLOT_CH):
        bp[r, j, 32 * j + r // SLOT] = 1.0
    return np.ascontiguousarray(bp.reshape(128, SLOT_CH * 128)).astype(BF_NP)


def _prep_quads(cls: np.ndarray, nq_slots: int):
    """Group same-class rows (sorted order) into quads.
    Returns (gidx [128, nq_slots/16] int16, q_global [n], qslot [n])."""
    n = cls.shape[0]
    order = np.argsort(cls, kind="stable")
    scls = cls[order].astype(np.int64)
    cnt = np.bincount(cls, minlength=VN)
    cstart = np.r_[0, np.cumsum(cnt)]
    rank = np.arange(n) - cstart[scls]
    nquad = (cnt + 3) // 4
    qbase = np.r_[0, np.cumsum(nquad)]
    q_of_sorted = qbase[scls] + (rank >> 2)
    nq_real = int(qbase[-1])
    assert nq_real <= nq_slots, (nq_real, nq_slots)
    qcls = np.zeros(nq_slots, np.int64)
    qcls[q_of_sorted] = scls
    # AllGather table layout: row == class id
    q_global = np.empty(n, np.int64)
    q_global[order] = q_of_sorted
    qslot = np.empty(n, np.int64)
    qslot[order] = rank & 3
    return _wrap_idx(qcls), q_global, qslot


def _make_plan(v_class, c_class):
    """(nq_slots, caps_v, caps_c): gather sizing + per-window slot caps."""
    nq_max = 0
    caps = {}
    for key, cls_all in (("v", v_class), ("c", c_class)):
        cap = np.zeros(WIN, np.int64)
        for core in range(N_CORES):
            cls = cls_all[core * SHARD:(core + 1) * SHARD]
            cnt = np.bincount(cls, minlength=VN)
            nq_max = max(nq_max, int(((cnt + 3) // 4).sum()))
            cap = np.maximum(cap, _slot_caps(cls))
        caps[key] = tuple(int(x) for x in cap)
    piece = CALLS_PER_PIECE * GC
    nq_slots = ((nq_max + piece - 1) // piece) * piece
    return nq_slots, caps["v"], caps["c"]


def _make_in_maps(v_s, c_s, v_sem, c_sem, v_class, c_class,
                  v_in_w, v_in_b, v_out_w, v_out_b,
                  c_in_w, c_in_b, c_out_w, c_out_b, nq_slots, caps_v, caps_c):
    v_class = np.asarray(v_class, np.int32)
    c_class = np.asarray(c_class, np.int32)
    v_bf = np.asarray(v_s, np.float32).astype(BF_NP)
    c_bf = np.asarray(c_s, np.float32).astype(BF_NP)
    v_semT = np.ascontiguousarray(np.asarray(v_sem, np.float32).T)
    c_semT = np.ascontiguousarray(np.asarray(c_sem, np.float32).T)
    wts = {"v": _branch_weights(v_in_w, v_in_b, v_out_w, v_out_b),
           "c": _branch_weights(c_in_w, c_in_b, c_out_w, c_out_b)}
    ident = np.eye(128, dtype=np.float32)
    iotaF = np.ascontiguousarray(
        np.broadcast_to(np.arange(128, dtype=np.float32), (128, 128))
    ).astype(BF_NP)
    bpat = _blockpat()
    caps = {"v": caps_v, "c": caps_c}
    invT = {}
    for br, cls in (("v", v_class), ("c", c_class)):
        cnt = np.bincount(cls, minlength=VN).astype(np.float32)
        inv = (1.0 / (cnt + 1e-8)).astype(np.float32)
        invT[br] = np.ascontiguousarray(
            np.broadcast_to(inv[None, :], (E, VN)))
    in_maps = []
    unmaps = []
    for core in range(N_CORES):
        b0 = core * SHARD
        m = {"ident": ident, "iotaF": iotaF, "bpat": bpat}
        um = {}
        for br, s_bf, cls_all, semT in (
                ("v", v_bf, v_class, v_semT), ("c", c_bf, c_class, c_semT)):
            cls = cls_all[b0:b0 + SHARD]
            sdat, cls2, _ns = _prep_phase_a(s_bf[b0:b0 + SHARD], cls, caps[br])
            gidx, q_global, qslot = _prep_quads(cls, nq_slots)
            m[f"sdat_{br}"] = sdat
            m[f"crel2_{br}"] = _prep_crel2(cls2, caps[br])
            m[f"gidx_{br}"] = gidx
            m[f"invT_{br}"] = invT[br]
            m[f"semq_{br}"] = np.ascontiguousarray(
                semT[:, core * QCH:(core + 1) * QCH]).astype(BF_NP)
            for k, vv in wts[br].items():
                m[f"{k}_{br}"] = vv
            um[br] = (q_global, qslot)
        in_maps.append(m)
        unmaps.append(um)
    return in_maps, unmaps


def _unpack_out(res_results, unmaps, nq_slots):
    """out_ext [2, n_gpiece, 128, CP*8, QELEM] bf16 -> full f32 outputs."""
    n_gpiece = nq_slots // (CALLS_PER_PIECE * GC)
    v_out = np.empty((NV, E), np.float32)
    c_out = np.empty((NV, E), np.float32)
    for core in range(N_CORES):
        o = np.asarray(res_results[core]["out"])
        if o.dtype != BF_NP:
            o = o.view(BF_NP)
        o = o.reshape(2, n_gpiece, 128, CALLS_PER_PIECE, GC // 128, QELEM)
        # quad q = piece*CP*GC + ci*GC + jj*128 + p  at o[br, piece, p, ci, jj]
        quads = np.ascontiguousarray(
            o.transpose(0, 1, 3, 4, 2, 5)).reshape(
                2, nq_slots, 4, E).astype(np.float32)
        for br_i, (br, out) in enumerate((("v", v_out), ("c", c_out))):
            q_global, qslot = unmaps[core][br]
            out[core * SHARD:(core + 1) * SHARD] = \
                quads[br_i, q_global, qslot, :]
    return v_out, c_out


def kernel(v_s, c_s, v_sem, c_sem, v_class, c_class,
           v_in_w, v_in_b, v_out_w, v_out_b,
           c_in_w, c_in_b, c_out_w, c_out_b):
    nq_slots, caps_v, caps_c = _make_plan(np.asarray(v_class, np.int32),
                                          np.asarray(c_class, np.int32))
    in_maps, unmaps = _make_in_maps(
        v_s, c_s, v_sem, c_sem, v_class, c_class,
        v_in_w, v_in_b, v_out_w, v_out_b,
        c_in_w, c_in_b, c_out_w, c_out_b, nq_slots, caps_v, caps_c)
    nc = _compiled(nq_slots, caps_v, caps_c)
    res = run_bass_kernel_spmd(nc, in_maps, core_ids=list(range(N_CORES)))
    return _unpack_out(res.results, unmaps, nq_slots)


# exposed for test.py timing
def prepare_in_maps(inputs):
    sig = ["v_s", "c_s", "v_sem", "c_sem", "v_class", "c_class",
           "v_in_w", "v_in_b", "v_out_w", "v_out_b",
           "c_in_w", "c_in_b", "c_out_w", "c_out_b"]
    kw = {k: inputs[k] for k in sig}
    nq_slots, caps_v, caps_c = _make_plan(np.asarray(kw["v_class"], np.int32),
                                          np.asarray(kw["c_class"], np.int32))
    in_maps, _ = _make_in_maps(**kw, nq_slots=nq_slots,
                               caps_v=caps_v, caps_c=caps_c)
    return _compiled(nq_slots, caps_v, caps_c), in_maps
